# revision 1
# baseline (speedup 1.0000x reference)
"""Two-layer GCN (GCNConv x2 + ReLU) on 8 Trainium2 NeuronCores.

Strategy: partition nodes by destination across the 8 cores. Each core:
  1. computes the full H1 = X @ W1 table (replicated; avoids a collective),
  2. aggregates its 1/8 of destination nodes over its incident edges using
     one-hot matmuls accumulated in PSUM (exact fp32 scatter-add),
  3. AllGathers the layer-1 activations H2 across cores (split into row
     slices so layer 2 can start on early slices while later ones transfer),
  4. aggregates layer 2 the same way, then applies W2 + b2.
Edge gathers use the SWDGE dma_gather instruction (int16 indices). Node
feature tables are split into SLICES separate tensors of <=32k rows each so
indices fit int16; the host groups each dst-block's edges by source slice.
Tables and messages are bf16; all accumulation is fp32 in PSUM.
"""
import sys
sys.path.insert(0, '/opt/trn_rl_repo')
import numpy as np
import concourse.bass as bass
import concourse.bacc as bacc
import concourse.mybir as mybir
import bass_rust
from concourse.tile import TileContext
from concourse.tile_rust import add_dep_helper
from concourse.bass_utils import run_bass_kernel_spmd

dt = mybir.dt

NCORES = 8
SLICES = 4          # table row-slice count (separate tensors + AllGathers)
MAXG = 8            # SWDGE ring caps one dma_gather at 1024 indices
TAB_DT = dt.bfloat16   # table/message/one-hot dtype
XW_DT = dt.bfloat16    # X@W1 input dtype


def _np_dt(d):
    return mybir.dt.np(d)


# ---------------------------------------------------------------------------
# walrus in this toolchain rejects >1 attached sem wait on several opcodes;
# hoist extras into standalone InstEventSemaphore instructions just before.
def hoist_excess_waits(nc, max_attached=1):
    n_new = 0
    for f in nc.m.functions:
        for bb in f.blocks:
            insts = bb.instructions  # live list
            i = 0
            while i < len(insts):
                inst = insts[i]
                si = inst.sync_info
                if si is not None and inst.engine is not None:
                    waits = list(si.on_wait)
                    imm = [w for w in waits if w.wait_reg is None]
                    other = [w for w in waits if w.wait_reg is not None]
                    budget = max_attached - len(other)
                    if len(imm) > budget:
                        if budget > 0:
                            extra, keep = imm[:-budget], imm[-budget:]
                        else:
                            extra, keep = imm, []
                        for w in extra:
                            ev = mybir.InstEventSemaphore(
                                name=f"I-hoistw{n_new}", ins=[], outs=[])
                            ev.engine = inst.engine
                            h = bass_rust.SemaphoreHandle(name=w.ant_name, num=w.id)
                            bass_rust.wait_op(ev, h, w.wait_value, "sem-ge", True)
                            insts.insert(i, ev)
                            i += 1
                            n_new += 1
                        si.on_wait = other + keep
                i += 1
    return n_new


# ---------------------------------------------------------------------------
# host-side graph preprocessing
def _prepare(x, edge_index, ncores):
    N, D = x.shape
    src0 = edge_index[0].astype(np.int64)
    dst0 = edge_index[1].astype(np.int64)
    loops = np.arange(N, dtype=np.int64)
    src = np.concatenate([src0, loops])
    dst = np.concatenate([dst0, loops])

    deg = np.bincount(dst, minlength=N).astype(np.float32)
    dinv = 1.0 / np.sqrt(np.maximum(deg, 1.0))
    norm = (dinv[src] * dinv[dst]).astype(np.float32)

    NSH = (N + ncores - 1) // ncores            # nodes per shard (6250)
    TS = (NSH + 127) // 128                     # dst blocks per shard (49)
    # stage-group size: largest divisor of TS that is <= 8
    SG = max(s for s in range(1, 9) if TS % s == 0)
    NSHP = TS * 128                             # padded shard rows (6272)
    NPAD = ncores * NSHP
    # asymmetric slice bounds (shard rows): a smaller first slice lets its
    # AllGather start earlier; bounds are multiples of SG for the store split.
    GPR = NSHP // (SG * 128)
    if GPR >= SLICES:
        # distribute stage groups, extras to the earliest slices (their
        # AllGathers overlap layer-1 work; later ones gate the critical path)
        base, rem = GPR // SLICES, GPR % SLICES
        parts = [base + (1 if i < rem else 0) for i in range(SLICES)]
        BOUNDS = [0]
        for p in parts:
            BOUNDS.append(BOUNDS[-1] + p * SG * 128)
    else:
        step = max(SG, ((NSHP // SLICES) // SG) * SG)
        BOUNDS = [min(i * step, NSHP) for i in range(SLICES)] + [NSHP]
    RSLS = [BOUNDS[i + 1] - BOUNDS[i] for i in range(SLICES)]
    assert all(r > 0 and r % SG == 0 for r in RSLS)
    assert all(ncores * r <= 32768 for r in RSLS)

    # table row layout chosen so device-side stores are large contiguous DMAs:
    # node n -> shard c = n//NSH, shard-row r = g*(SG*128) + p*SG + s
    # (t = l//128 = g*SG+s, p = l%128); slice j = r//RSL holds table row
    # c*RSL + (r - j*RSL).
    def rowmap_shard(n):
        l = n % NSH
        t, p = l // 128, l % 128
        g, s = t // SG, t % SG
        return n // NSH, g * (SG * 128) + p * SG + s

    src_c, src_r = rowmap_shard(src)
    slice_flag = np.searchsorted(np.array(BOUNDS), src_r, side='right') - 1
    src_tab = (src_c * np.array(RSLS)[slice_flag]
               + (src_r - np.array(BOUNDS)[slice_flag]))

    dst_loc = dst % NSH
    dst_blk = dst_loc // 128
    dst_in_blk = dst_loc % 128
    dst_core = dst // NSH

    # per-(core, block, slice) edge lists
    groups = {}
    for c in range(ncores):
        eidx = np.nonzero(dst_core == c)[0]
        b_arr, j_arr = dst_blk[eidx], slice_flag[eidx]
        for b in range(TS):
            mb = b_arr == b
            for j in range(SLICES):
                groups[(c, b, j)] = eidx[mb & (j_arr == j)]

    # chunk counts per (block, slice): max over cores (shared program)
    m_cnt = {}
    for b in range(TS):
        for j in range(SLICES):
            mx = max(len(groups[(c, b, j)]) for c in range(ncores))
            m_cnt[(b, j)] = (mx + 127) // 128

    # flat chunk order: for G (super-group of SG blocks): for j: for b in G
    runs = []       # (j, [(b, m, chunk_off), ...])
    blk_first = {}
    blk_last = {}
    off = 0
    for G in range(TS // SG):
        for j in range(SLICES):
            blocks = []
            for b in range(G * SG, (G + 1) * SG):
                m = m_cnt[(b, j)]
                if m == 0:
                    continue
                for k in range(off, off + m):
                    if b not in blk_first:
                        blk_first[b] = k
                    blk_last[b] = k
                blocks.append((b, m, off))
                off += m
            if blocks:
                runs.append((j, blocks))
    NCHT = off

    # per-core flat edge arrays in chunk order, padded with null edges
    idx_np = np.zeros((ncores, 128, NCHT * 8), np.int16)
    dstl_np = np.zeros((ncores, 128, NCHT), np.float32)
    norm_np = np.zeros((ncores, 128, NCHT), np.float32)
    for c in range(ncores):
        flat_src = np.zeros(NCHT * 128, np.int64)
        flat_dst = np.zeros(NCHT * 128, np.float32)
        flat_nrm = np.zeros(NCHT * 128, np.float32)
        for (j, blocks) in runs:
            for (b, m, o) in blocks:
                e = groups[(c, b, j)]
                n = len(e)
                flat_src[o * 128:o * 128 + n] = src_tab[e]
                flat_dst[o * 128:o * 128 + n] = dst_in_blk[e]
                flat_nrm[o * 128:o * 128 + n] = norm[e]
        i16 = flat_src.astype(np.int16).reshape(-1, 16).T      # [16, NCHT*8]
        idx_np[c] = np.tile(i16, (8, 1))
        dstl_np[c] = flat_dst.reshape(NCHT, 128).T
        norm_np[c] = flat_nrm.reshape(NCHT, 128).T

    # xT columns in plain padded per-shard node order; the XW phase's staged
    # store applies the p/s permutation that lands rows at rowmap(n).
    nn = np.arange(N, dtype=np.int64)
    colmap = (nn // NSH) * NSHP + (nn % NSH)
    xT = np.zeros((D, NPAD), np.float32)
    xT[:, colmap] = x.T

    iota = np.tile(np.arange(128, dtype=np.float32)[None, :], (128, 1)).copy()

    return dict(N=N, D=D, NSH=NSH, TS=TS, SG=SG, NSHP=NSHP, NPAD=NPAD,
                BOUNDS=BOUNDS, RSLS=RSLS, NCHT=NCHT, runs=runs,
                blk_first=blk_first, blk_last=blk_last,
                idx_np=idx_np, dstl_np=dstl_np, norm_np=norm_np,
                xT=xT, iota=iota)


# ---------------------------------------------------------------------------
def _build(cfg, F1, F2, debug=False):
    NPAD, NSHP = cfg['NPAD'], cfg['NSHP']
    BOUNDS, RSLS = cfg['BOUNDS'], cfg['RSLS']
    D, TS, SG, NCHT = cfg['D'], cfg['TS'], cfg['SG'], cfg['NCHT']
    runs, blk_first, blk_last = cfg['runs'], cfg['blk_first'], cfg['blk_last']
    KD = D // 128
    GROUPS = NPAD // (SG * 128)     # XW stage groups over the whole table
    SGROWS = SG * 128
    GPS = GROUPS // NCORES          # XW stage groups per shard (= TS//SG)

    nc = bacc.Bacc(None, target_bir_lowering=False)
    xT_d = nc.declare_dram_parameter("xT", [D, NPAD], XW_DT, isOutput=False)
    W1_d = nc.declare_dram_parameter("W1", [D, F1], XW_DT, isOutput=False)
    b1_d = nc.declare_dram_parameter("b1", [F1, 1], dt.float32, isOutput=False)
    W2_d = nc.declare_dram_parameter("W2", [F1, F2], TAB_DT, isOutput=False)
    b2_d = nc.declare_dram_parameter("b2", [F2, 1], dt.float32, isOutput=False)
    iota_d = nc.declare_dram_parameter("iota", [128, 128], TAB_DT, isOutput=False)
    ones1_d = nc.declare_dram_parameter("ones1", [1, 128], dt.float32, isOutput=False)
    idx_d = nc.declare_dram_parameter("idx", [128, NCHT * 8], dt.int16, isOutput=False)
    dstl_d = nc.declare_dram_parameter("dstl", [128, NCHT], dt.float32, isOutput=False)
    norm_d = nc.declare_dram_parameter("norm", [128, NCHT], dt.float32, isOutput=False)
    out_d = nc.declare_dram_parameter("outT", [F2, NSHP], dt.float32, isOutput=True)

    # per-slice node-feature tables, [8*RSL, F1] each (row = c*RSL + r)
    H1tabs = [nc.dram_tensor(f"H1tab{j}", [NCORES * RSLS[j], F1], TAB_DT)
              for j in range(SLICES)]
    H2shs = [nc.dram_tensor(f"H2sh{j}", [RSLS[j], F1], TAB_DT)
             for j in range(SLICES)]
    H2tabs = [nc.dram_tensor(f"H2tab{j}", [NCORES * RSLS[j], F1], TAB_DT,
                             addr_space="Shared") for j in range(SLICES)]
    if debug:
        H1dbg = nc.declare_dram_parameter("H1dbg", [NPAD, F1], TAB_DT, isOutput=True)
        H2dbg = nc.declare_dram_parameter("H2dbg", [NPAD, F1], TAB_DT, isOutput=True)

    with TileContext(nc) as tc:
        with (
            tc.tile_pool(name="const", bufs=1) as cp,
            tc.tile_pool(name="xw", bufs=3) as xp,
            tc.tile_pool(name="gat", bufs=4) as gp,
            tc.tile_pool(name="oh", bufs=6) as ohp,
            tc.tile_pool(name="evac", bufs=3) as evp,
        ):
            # ---- constants / metadata resident in SBUF ----
            iota_t = cp.tile([128, 128], TAB_DT, tag="iota")
            nc.sync.dma_start(iota_t[:], iota_d[:])
            ones1_t = cp.tile([1, 128], dt.float32, tag="ones1")
            nc.sync.dma_start(ones1_t[:], ones1_d[:])
            b1r_t = cp.tile([1, F1], dt.float32, tag="b1r")
            nc.sync.dma_start(b1r_t[:], b1_d[:].rearrange("f one -> one f"))
            b2_t = cp.tile([F2, 1], dt.float32, tag="b2")
            nc.sync.dma_start(b2_t[:], b2_d[:])
            W1_t = cp.tile([D if KD == 1 else 128, KD, F1], XW_DT, tag="W1")
            nc.sync.dma_start(W1_t[:], W1_d[:].rearrange("(k p) f -> p k f", p=128 if KD > 1 else D))
            W2_t = cp.tile([F1, F2], TAB_DT, tag="W2")
            nc.sync.dma_start(W2_t[:], W2_d[:])
            idx_t = cp.tile([128, NCHT * 8], dt.int16, tag="idx")
            nc.sync.dma_start(idx_t[:], idx_d[:])
            dstl_t = cp.tile([128, NCHT], dt.float32, tag="dstl")
            nc.sync.dma_start(dstl_t[:], dstl_d[:])
            norm_t = cp.tile([128, NCHT], dt.float32, tag="norm")
            nc.sync.dma_start(norm_t[:], norm_d[:])

            def store_group_rows(tensors, stage, c, g_in_shard, nrows_per_p):
                """DMA a staged [P, nrows_per_p, F] tile into the per-slice
                tables. Shard rows covered: r0 + p*nrows_per_p + s. Split on
                the partition dim at slice boundaries. Returns [(j, inst)].
                `tensors[j]` row base is c*RSL (c=None for shard-local)."""
                r0 = g_in_shard * 128 * nrows_per_p
                out = []
                for j in range(SLICES):
                    p0 = max(0, (BOUNDS[j] - r0)) // nrows_per_p
                    p1 = min(128, max(0, BOUNDS[j + 1] - r0) // nrows_per_p)
                    if p1 <= p0:
                        continue
                    base = (0 if c is None else c * RSLS[j]) + r0 - BOUNDS[j]
                    w = nc.sync.dma_start(
                        tensors[j][base + p0 * nrows_per_p:
                                   base + p1 * nrows_per_p, :]
                        .rearrange("(p s) f -> p s f", s=nrows_per_p),
                        stage[p0:p1, :, :])
                    out.append((j, w))
                return out

            # ---- phase 1: H1 = X @ W1, full table, node(-row)-major ----
            # emit slice-major so layer-1 slice-j gathers unblock early
            xw_writes = {j: [] for j in range(SLICES)}
            xw_order = []
            for jj in range(SLICES):
                for gg in range(GPS):
                    if max(i for i in range(SLICES)
                           if BOUNDS[i] <= gg * SGROWS) == jj:
                        for c in range(NCORES):
                            xw_order.append(c * GPS + gg)
            assert sorted(xw_order) == list(range(GROUPS))
            with tc.tile_pool(name="xwps", bufs=4, space="PSUM") as xpp:
                for g in xw_order:
                    xt = xp.tile([128, KD, SGROWS], XW_DT, tag="xt")
                    nc.sync.dma_start(
                        xt[:],
                        xT_d[:, g * SGROWS:(g + 1) * SGROWS].rearrange(
                            "(k p) n -> p k n", p=128 if KD > 1 else D))
                    stage = xp.tile([128, SG, F1], TAB_DT, tag="h1stage")
                    for s in range(SG):
                        ps = xpp.tile([128, F1], dt.float32, tag="xwps")
                        for k in range(KD):
                            nc.tensor.matmul(ps[:], xt[:, k, s * 128:(s + 1) * 128],
                                             W1_t[:, k, :],
                                             start=(k == 0), stop=(k == KD - 1))
                        nc.vector.tensor_copy(stage[:, s, :], ps[:])
                    for (j, w) in store_group_rows(H1tabs, stage,
                                                   g // GPS, g % GPS, SG):
                        xw_writes[j].append(w)

            # ---- shared aggregation emitter over the chunk schedule ----
            # node_major=True  -> acc[dst, f]  (lhsT=onehot, rhs=msgs)
            # node_major=False -> acc[f, dst]  (lhsT=msgs, rhs=onehot)
            def agg_layer(tabs, node_major, gather_deps, psum_pool,
                          after_group=None, extra_mms=0, runs_subset=None,
                          acc_tag="acc"):
                my_runs = runs if runs_subset is None else runs_subset
                last_of_blk = {}
                for (j, blocks) in my_runs:
                    for (b, m, o) in blocks:
                        last_of_blk[b] = o + m - 1
                accs = {}
                done_in_blk = {}

                def get_acc(b):
                    if b not in accs:
                        shape = [128, F1] if node_major else [F1, 128]
                        accs[b] = psum_pool.tile(shape, dt.float32,
                                                 name=f"{acc_tag}{b}", tag=acc_tag)
                        done_in_blk[b] = 0
                    return accs[b]

                cur_G = my_runs[0][1][0][0] // SG
                for (j, blocks) in my_runs:
                    G = blocks[0][0] // SG
                    if G != cur_G:
                        if after_group is not None:
                            after_group(cur_G, accs)
                        cur_G = G
                    chunk_list = [(b, k) for (b, m, o) in blocks
                                  for k in range(o, o + m)]
                    for w0 in range(0, len(chunk_list), MAXG):
                        win = chunk_list[w0:w0 + MAXG]
                        o = win[0][1]
                        m = len(win)
                        gt = gp.tile([128, MAXG, F1], TAB_DT, tag="gat")
                        gi = nc.gpsimd.dma_gather(
                            gt[:, 0:m, :], tabs[j][:], idx_t[:, o * 8:(o + m) * 8],
                            num_idxs=m * 128, num_idxs_reg=m * 128, elem_size=F1)
                        for dep in gather_deps[j]:
                            add_dep_helper(gi.ins, dep.ins, reason="gather table dep")
                        for slot, (b, k) in enumerate(win):
                            acc = get_acc(b)
                            oh = ohp.tile([128, 128], TAB_DT, tag="oh")
                            nc.vector.tensor_scalar(
                                oh[:], iota_t[:], dstl_t[:, k:k + 1],
                                norm_t[:, k:k + 1],
                                mybir.AluOpType.is_equal, mybir.AluOpType.mult)
                            first = (done_in_blk[b] == 0)
                            done_in_blk[b] += 1
                            last = (k == last_of_blk[b]) and extra_mms == 0
                            if node_major:
                                nc.tensor.matmul(acc[:], oh[:], gt[:, slot, :],
                                                 start=first, stop=last)
                            else:
                                nc.tensor.matmul(acc[:], gt[:, slot, :], oh[:],
                                                 start=first, stop=last)
                if after_group is not None:
                    after_group(cur_G, accs)
                return accs

            # ---- layer 1: aggregate (node-major), +b1 via K=1 matmul, relu ----
            h2_writes = {j: [] for j in range(SLICES)}
            ag_deps = {}

            def l1_after_group(g, accs1):
                stage = evp.tile([128, SG, F1], TAB_DT, tag="h2stage")
                for s in range(SG):
                    b = g * SG + s
                    nc.tensor.matmul(accs1[b][:], ones1_t[:], b1r_t[:],
                                     start=False, stop=True)
                    nc.scalar.activation(stage[:, s, :], accs1[b][:],
                                         mybir.ActivationFunctionType.Relu,
                                         bias=0.0, scale=1.0)
                for (j, w) in store_group_rows(H2shs, stage, None, g, SG):
                    h2_writes[j].append(w)
                # AllGather slice j right after its last stage group's write,
                # so it sits early in the Pool queue and overlaps the rest of
                # layer 1 on the collective cores.
                for j in range(SLICES):
                    if g == (BOUNDS[j + 1] - 1) // SGROWS:
                        cc = nc.gpsimd.collective_compute(
                            "AllGather", mybir.AluOpType.bypass,
                            replica_groups=[list(range(NCORES))],
                            ins=[H2shs[j][:]], outs=[H2tabs[j][:]])
                        for w in h2_writes[j]:
                            add_dep_helper(cc.ins, w.ins,
                                           reason="allgather reads H2 slice")
                        ag_deps[j] = [cc]

            with tc.tile_pool(name="aggps1", bufs=SG, space="PSUM") as app1:
                agg_layer(H1tabs, True, xw_writes, app1,
                          after_group=l1_after_group, extra_mms=1)


            # ---- layer 2: aggregate H2 (feature-major), then W2 + b2 ----
            with (
                tc.tile_pool(name="aggps2", bufs=SG, space="PSUM") as app2,
                tc.tile_pool(name="w2ps", bufs=1, space="PSUM") as wpp,
                tc.tile_pool(name="part2", bufs=2) as p2pool,
            ):
                # N-pass layer 2: pass si aggregates slice-si edges into
                # PSUM; non-final passes park/merge partials in SBUF so the
                # work overlaps the later slices' AllGathers; the final pass
                # merges and applies W2 + b2.
                part2 = {}

                def l2_mid_after(g, accs2):
                    for s in range(SG):
                        b = g * SG + s
                        if b not in accs2:
                            continue
                        pt = p2pool.tile([F1, 128], TAB_DT,
                                         name=f"part2_{b}", tag=f"p2_{b}")
                        if b in part2:
                            nc.vector.tensor_tensor(
                                pt[:], accs2[b][:], part2[b][:],
                                mybir.AluOpType.add)
                        else:
                            nc.scalar.activation(pt[:], accs2[b][:],
                                                 mybir.ActivationFunctionType.Copy)
                        part2[b] = pt

                def l2_final_after(g, accs2):
                    ostage = evp.tile([F2, SG, 128], dt.float32, tag="ostage")
                    for s in range(SG):
                        b = g * SG + s
                        m2 = evp.tile([F1, 128], TAB_DT, name=f"m2_{b}", tag="m2")
                        if b in accs2 and b in part2:
                            nc.vector.tensor_tensor(
                                m2[:], accs2[b][:], part2[b][:],
                                mybir.AluOpType.add)
                        elif b in accs2:
                            nc.scalar.activation(m2[:], accs2[b][:],
                                                 mybir.ActivationFunctionType.Copy)
                        else:
                            nc.vector.tensor_copy(m2[:], part2[b][:])
                        p2 = wpp.tile([F2, 128], dt.float32, tag="w2ps")
                        nc.tensor.matmul(p2[:], W2_t[:], m2[:],
                                         start=True, stop=True)
                        nc.scalar.activation(ostage[:, s, :], p2[:],
                                             mybir.ActivationFunctionType.Identity,
                                             bias=b2_t[:, 0:1], scale=1.0)
                    nc.sync.dma_start(
                        out_d[:, g * SGROWS:(g + 1) * SGROWS].rearrange(
                            "f (s n) -> f s n", s=SG),
                        ostage[:])

                for si in range(SLICES):
                    runs_si = [r for r in runs if r[0] == si]
                    agg_layer(H2tabs, False, ag_deps, app2,
                              after_group=(l2_final_after if si == SLICES - 1
                                           else l2_mid_after),
                              runs_subset=runs_si)

                if debug:
                    tc.strict_bb_all_engine_barrier()
                    for j in range(SLICES):
                        for c in range(NCORES):
                            r0 = c * NSHP + BOUNDS[j]
                            nc.sync.dma_start(
                                H1dbg[r0:r0 + RSLS[j], :],
                                H1tabs[j][c * RSLS[j]:(c + 1) * RSLS[j], :])
                            nc.sync.dma_start(
                                H2dbg[r0:r0 + RSLS[j], :],
                                H2tabs[j][c * RSLS[j]:(c + 1) * RSLS[j], :])

    if not nc.is_finalized():
        nc.finalize()
    hoist_excess_waits(nc)
    return nc


# ---------------------------------------------------------------------------
def _kernel_impl(x, edge_index, W1, b1, W2, b2, ncores=NCORES, debug=False):
    x = np.asarray(x, dtype=np.float32)
    edge_index = np.asarray(edge_index)
    W1 = np.asarray(W1, dtype=np.float32)
    b1 = np.asarray(b1, dtype=np.float32)
    W2 = np.asarray(W2, dtype=np.float32)
    b2 = np.asarray(b2, dtype=np.float32)
    N, D = x.shape
    F1 = W1.shape[1]
    F2 = W2.shape[1]

    cfg = _prepare(x, edge_index, ncores)
    nc = _build(cfg, F1, F2, debug=debug)

    xwnp = _np_dt(XW_DT)
    tabnp = _np_dt(TAB_DT)
    in_maps = []
    for c in range(ncores):
        in_maps.append({
            "xT": cfg['xT'].astype(xwnp),
            "W1": W1.astype(xwnp),
            "b1": b1.reshape(F1, 1).astype(np.float32),
            "W2": W2.astype(tabnp),
            "b2": b2.reshape(F2, 1).astype(np.float32),
            "iota": cfg['iota'].astype(tabnp),
            "ones1": np.ones((1, 128), np.float32),
            "idx": cfg['idx_np'][c],
            "dstl": cfg['dstl_np'][c],
            "norm": cfg['norm_np'][c],
        })
    res = run_bass_kernel_spmd(nc, in_maps, list(range(ncores)))

    NSH = cfg['NSH']
    out = np.empty((N, F2), np.float32)
    # outT columns are plain shard-local node order (col = t*128 + p = l)
    for c in range(ncores):
        oT = res.results[c]["outT"]          # [F2, NSHP]
        n0 = c * NSH
        n1 = min(N, n0 + NSH)
        out[n0:n1] = oT[:, :n1 - n0].T
    return out, res, nc, cfg


def kernel(x, edge_index, W1, b1, W2, b2):
    out, _, _, _ = _kernel_impl(x, edge_index, W1, b1, W2, b2)
    return out

